# revision 9
# baseline (speedup 1.0000x reference)
"""Trainium2 Bass kernel for nn_MemNet (memory-network attention block).

Computation (per row r of B*R=5120 rows):
    fused  = tanh(cat(img, ques) @ W_fuse.T + b_fuse)          [5120, 512]
    s_j    = sum_d hist[r,j,d] * fused[r,d] * w_att[d] + b_att [5120, 10]
    attn   = softmax(s, axis=1)
    he     = sum_j attn[r,j] * hist[r,j,:]                     [5120, 512]
    he     = tanh(he @ W_hist.T + b_hist)
    out    = fused + he   -> reshape [512, 10, 512]

Strategy: pure data parallel over the leading 5120 rows -> 640 rows/core on
8 cores, 5 row-tiles of 128 rows each.  Weights replicated.  Activations for
the big matmul are pre-transposed on the host so the contraction dim lands on
SBUF partitions (no on-chip transposes for matmul 1).  The attention middle
stays in natural [row, feature] layout and runs on the Vector engine with
fused ops (tensor_tensor_reduce / scalar_tensor_tensor).  hist_embed is
transposed on the PE (4x 128x128) to feed matmul 2.  Biases are folded into
the PSUM accumulation via a ones-row matmul.
"""

import os

import numpy as np

# ---- problem constants (hardcoded per contract) ----
B = 512
R = 10
BR = B * R  # 5120
IMG = 2048
D = 512
FUSION = IMG + D  # 2560
NCORES = 8
ROWS = BR // NCORES  # 640
NRT = ROWS // 128  # 5 row tiles / core
KC = FUSION // 128  # 20 contraction chunks for matmul1
DC = D // 128  # 4 contraction chunks for matmul2

# packed-constants column offsets (floats per partition)
OFF_WH = 0
OFF_WATT = OFF_WH + DC * D  # 2048
OFF_EYE = OFF_WATT + D  # 2560
OFF_BFUSE = OFF_EYE + 128  # 2688
OFF_BHIST = OFF_BFUSE + D  # 3200
OFF_ONES = OFF_BHIST + D  # 3712
OFF_BATT = OFF_ONES + 128  # 3840
CCOLS = OFF_BATT + 1  # 3841

# matmul dtype: "fp32r" (full-rate fp32, slight precision differences on HW)
# or "fp32" (4x slower on PE, bit-accurate).
MM_DTYPE = os.environ.get("MEMNET_MM_DTYPE", "fp32r")

_PROGRAM = None
LAST_RESULTS = None  # BassKernelResults of the most recent run (for profiling)


def _build_program():
    import concourse.bacc as bacc
    import concourse.mybir as mybir
    import concourse.tile as tile

    dt = mybir.dt
    f32 = dt.float32
    Alu = mybir.AluOpType
    Act = mybir.ActivationFunctionType
    Ax = mybir.AxisListType

    # matmul1 operands live as float32r end-to-end (DRAM + SBUF) so the
    # BIR verifier sees fp32r-rounded producers for the fp32r matmul.
    mmdt = dt.float32r if MM_DTYPE == "fp32r" else f32

    nc = bacc.Bacc("TRN2", target_bir_lowering=False, debug=False)

    # per-core inputs.  All small f32 constants travel in one packed DMA so
    # they complete on a single DMA semaphore lane (instructions have very
    # few sync-wait slots; LW-bearing matmuls have exactly one).
    fvt = nc.dram_tensor("fvt", [NRT, 128, KC, 128], mmdt, kind="ExternalInput")
    hist = nc.dram_tensor("hist", [ROWS, R, D], f32, kind="ExternalInput")
    w1 = nc.dram_tensor("w1", [128, KC, D], mmdt, kind="ExternalInput")
    cpack = nc.dram_tensor("cpack", [128, CCOLS], f32, kind="ExternalInput")
    out = nc.dram_tensor("out", [ROWS, D], f32, kind="ExternalOutput")

    with tile.TileContext(nc) as tc:
        with (
            tc.tile_pool(name="const", bufs=1) as cpool,
            tc.tile_pool(name="act", bufs=2) as apool,
            tc.tile_pool(name="histp", bufs=2) as hpool,
            tc.tile_pool(name="work", bufs=2) as wpool,
            tc.tile_pool(name="small", bufs=2) as spool,
            tc.tile_pool(name="ps1", bufs=2, space="PSUM") as pp1,
            tc.tile_pool(name="pst", bufs=2, space="PSUM") as ppt,
            tc.tile_pool(name="ps2", bufs=2, space="PSUM") as pp2,
            tc.tile_pool(name="psj", bufs=1, space="PSUM") as ppj,
        ):
            w1_sb = cpool.tile([128, KC, D], mmdt)
            nc.sync.dma_start(w1_sb[:], w1[:])
            cp_sb = cpool.tile([128, CCOLS], f32)
            nc.sync.dma_start(cp_sb[:], cpack[:])

            def wh_ap(c):
                return cp_sb[:, OFF_WH + c * D : OFF_WH + (c + 1) * D]

            watt_ap = cp_sb[:, OFF_WATT : OFF_WATT + D]
            eye_ap = cp_sb[:, OFF_EYE : OFF_EYE + 128]
            bfuse_ap = cp_sb[0:1, OFF_BFUSE : OFF_BFUSE + D]
            bhist_ap = cp_sb[0:1, OFF_BHIST : OFF_BHIST + D]
            ones_ap = cp_sb[0:1, OFF_ONES : OFF_ONES + 128]
            batt_ap = cp_sb[:, OFF_BATT : OFF_BATT + 1]

            # wait-absorbers: one PE op takes the w1 DMA wait, one DVE op
            # takes the cpack DMA wait; later ops then carry <=1 wait each.
            junk_ps = ppj.tile([128, 1], f32)
            nc.tensor.matmul(
                junk_ps[:],
                w1_sb[:, 0, :128].bitcast(f32),
                w1_sb[:, 0, 0:1].bitcast(f32),
                start=True,
                stop=True,
            )
            absorb = spool.tile([128, 1], f32, tag="absorb")
            nc.vector.tensor_copy(absorb[:], cp_sb[:, 0:1])

            for rt in range(NRT):
                a_sb = apool.tile([128, KC, 128], mmdt, tag="a")
                nc.sync.dma_start(a_sb[:], fvt[rt])
                h_sb = hpool.tile([128, R, D], f32, tag="h")
                nc.sync.dma_start(h_sb[:], hist[rt * 128 : (rt + 1) * 128])

                # ---- matmul1: fused = tanh(fv @ W_fuse.T + b_fuse) ----
                # bias matmul leads the accumulation group: it carries the
                # PSUM WAW wait, so the k=0 matmul only waits on its DMA.
                ps1 = pp1.tile([128, D], f32, tag="ps1")
                nc.tensor.matmul(
                    ps1[:], ones_ap, bfuse_ap, start=True, stop=False
                )
                for k in range(KC):
                    nc.tensor.matmul(
                        ps1[:],
                        a_sb[:, k, :],
                        w1_sb[:, k, :],
                        start=False,
                        stop=(k == KC - 1),
                    )
                fused_sb = wpool.tile([128, D], f32, tag="fused")
                nc.scalar.activation(fused_sb[:], ps1[:], Act.Tanh)

                # ---- attention scores ----
                wfused_sb = wpool.tile([128, D], f32, tag="wfused")
                nc.vector.tensor_mul(wfused_sb[:], fused_sb[:], watt_ap)
                # scores_j = sum_d hist_j*wfused (b_att dropped: softmax is
                # shift-invariant so it cannot affect the output)
                scores = spool.tile([128, R], f32, tag="scores")
                scratch = wpool.tile([128, D], f32, tag="scratch")
                for j in range(R):
                    nc.vector.scalar_tensor_tensor(
                        out=scratch[:],
                        in0=h_sb[:, j, :],
                        scalar=0.0,
                        in1=wfused_sb[:],
                        op0=Alu.bypass,
                        op1=Alu.mult,
                        accum_out=scores[:, j : j + 1],
                    )

                # ---- softmax over the R=10 scores ----
                negmax = spool.tile([128, 1], f32, tag="negmax")
                nc.vector.reduce_max(negmax[:], scores[:], axis=Ax.X, negate=True)
                probs = spool.tile([128, R], f32, tag="probs")
                sumexp = spool.tile([128, 1], f32, tag="sumexp")
                nc.scalar.activation(
                    probs[:],
                    scores[:],
                    Act.Exp,
                    bias=negmax[:],
                    scale=1.0,
                    accum_out=sumexp[:],
                )
                rcp = spool.tile([128, 1], f32, tag="rcp")
                nc.vector.reciprocal(rcp[:], sumexp[:])
                attn = spool.tile([128, R], f32, tag="attn")
                nc.vector.tensor_scalar_mul(attn[:], probs[:], rcp[:])

                # ---- weighted sum of hist rows (ping-pong accumulate) ----
                acc_a = wpool.tile([128, D], f32, tag="acca")
                acc_b = wpool.tile([128, D], f32, tag="accb")
                nc.vector.tensor_scalar_mul(acc_a[:], h_sb[:, 0, :], attn[:, 0:1])
                cur, nxt = acc_a, acc_b
                for j in range(1, R):
                    nc.vector.scalar_tensor_tensor(
                        out=nxt[:],
                        in0=h_sb[:, j, :],
                        scalar=attn[:, j : j + 1],
                        in1=cur[:],
                        op0=Alu.mult,
                        op1=Alu.add,
                    )
                    cur, nxt = nxt, cur

                # ---- transpose hist_embed for matmul2 ----
                het_sb = wpool.tile([128, DC, 128], f32, tag="het")
                for c in range(DC):
                    pst = ppt.tile([128, 128], f32, tag="pst")
                    nc.tensor.transpose(
                        pst[:], cur[:, c * 128 : (c + 1) * 128], eye_ap
                    )
                    nc.vector.tensor_copy(het_sb[:, c, :], pst[:])

                # ---- matmul2: he = tanh(he @ W_hist.T + b_hist) ----
                ps2 = pp2.tile([128, D], f32, tag="ps2")
                nc.tensor.matmul(
                    ps2[:], ones_ap, bhist_ap, start=True, stop=False
                )
                for c in range(DC):
                    nc.tensor.matmul(
                        ps2[:],
                        het_sb[:, c, :],
                        wh_ap(c),
                        start=False,
                        stop=(c == DC - 1),
                    )
                he_sb = wpool.tile([128, D], f32, tag="he")
                nc.scalar.activation(he_sb[:], ps2[:], Act.Tanh)

                # ---- residual add + store ----
                out_sb = wpool.tile([128, D], f32, tag="out")
                nc.vector.tensor_add(out_sb[:], fused_sb[:], he_sb[:])
                nc.scalar.dma_start(out[rt * 128 : (rt + 1) * 128, :], out_sb[:])

    nc.compile()
    return nc


def get_program():
    global _PROGRAM
    if _PROGRAM is None:
        _PROGRAM = _build_program()
    return _PROGRAM


def shard_inputs(img, ques, hist, W_fuse, b_fuse, w_att, b_att, W_hist, b_hist):
    """Host-side layout preprocessing + sharding.  Returns list of in_maps."""
    f = np.float32
    img = np.asarray(img, f)
    ques = np.asarray(ques, f)
    hist = np.asarray(hist, f)
    W_fuse = np.asarray(W_fuse, f)
    W_hist = np.asarray(W_hist, f)

    fv = np.concatenate([img, ques], axis=1)  # [5120, 2560]
    # fvt[core][rt, p, c, r] = fv[core*640 + rt*128 + r, c*128 + p]
    fvt = np.ascontiguousarray(
        fv.reshape(NCORES, NRT, 128, KC, 128).transpose(0, 1, 4, 3, 2)
    )
    hist_sh = np.ascontiguousarray(hist.reshape(NCORES, ROWS, R, D))

    # w1[p, c, n] = W_fuse[n, c*128 + p]
    w1 = np.ascontiguousarray(W_fuse.T.reshape(KC, 128, D).transpose(1, 0, 2))

    cpack = np.zeros((128, CCOLS), f)
    # wh[p, c*D + n] = W_hist[n, c*128 + p]
    cpack[:, OFF_WH : OFF_WH + DC * D] = (
        W_hist.T.reshape(DC, 128, D).transpose(1, 0, 2).reshape(128, DC * D)
    )
    cpack[:, OFF_WATT : OFF_WATT + D] = np.asarray(w_att, f)[None, :]
    cpack[:, OFF_EYE : OFF_EYE + 128] = np.eye(128, dtype=f)
    cpack[:, OFF_BFUSE : OFF_BFUSE + D] = np.asarray(b_fuse, f)[None, :]
    cpack[:, OFF_BHIST : OFF_BHIST + D] = np.asarray(b_hist, f)[None, :]
    cpack[:, OFF_ONES : OFF_ONES + 128] = 1.0
    cpack[:, OFF_BATT] = float(np.asarray(b_att))

    return [
        {
            "fvt": fvt[c],
            "hist": hist_sh[c],
            "w1": w1,
            "cpack": cpack,
        }
        for c in range(NCORES)
    ]


def kernel(
    img,
    ques,
    hist,
    W_fuse,
    b_fuse,
    w_att,
    b_att,
    W_hist,
    b_hist,
    batch_size=B,
    num_rounds=R,
    **_unused,
):
    global LAST_RESULTS
    from concourse.bass_utils import run_bass_kernel_spmd

    nc = get_program()
    in_maps = shard_inputs(
        img, ques, hist, W_fuse, b_fuse, w_att, b_att, W_hist, b_hist
    )
    trace = bool(int(os.environ.get("MEMNET_TRACE", "0")))
    res = run_bass_kernel_spmd(
        nc, in_maps, core_ids=list(range(NCORES)), trace=trace
    )
    LAST_RESULTS = res
    full = np.concatenate([res.results[c]["out"] for c in range(NCORES)], axis=0)
    return full.reshape(B, R, D).astype(np.float32)
